# revision 11
# baseline (speedup 1.0000x reference)
"""Trainium2 Bass kernel for the 1-D Bessel (von Mises-like) kernel matrix:

    K[i, j] = I0(2a * cos(pi * (x_i - y_j))) * exp(-2a),   a = 10

Algorithm (pair-interpolated log-space rank-101 factorization)
--------------------------------------------------------------
log K has a rapidly converging Fourier cosine series in d = x - y:

    log K = b0 + sum_{k=1..31} b_k cos(2 pi k d)            (trunc err 1.6e-4)

so log K = U.T @ V with trig feature matrices of rank 63 (+38 bf16 hi/lo
correction rows -> K-dim 101, ONE bf16 matmul pass into fp32 PSUM).

To halve the Scalar-engine exp cost (the 1 elem/cycle/lane ACT floor), x is
sorted on host and adjacent rows are paired.  For each pair the device gets
the even row's features u(x_e) and the delta features u(x_o) - u(x_e), so
PSUM holds the even-row logs L_e and the exact pair deltas dL
(|dL| <= 0.058 on this data).  Then per 2048-col group:

    ACT:   out_even = exp(L_e)              (fp16, only HALF the rows)
    DVE:   out_odd  = (dL + 1) * out_even   (one fused scalar_tensor_tensor)

with linearization error dL^2/2 <= 1.7e-3 (gate 2e-2).  The 2^16 fp16
output scale is folded into the constant feature row; the host multiplies
by 2^-16 while un-sorting rows.  End-to-end max pointwise rel err ~2.5e-3.

Per-core busy estimates: PE 34us (128 bf16 matmuls), ACT 30us, DVE 38us,
and the 16 MiB fp16 output DMA ~41us at the 16-engine ~410 GB/s per-core
ceiling -- the kernel is output-DMA-bound by design.
"""

import os
import sys

import numpy as np

sys.path.insert(0, "/opt/trn_rl_repo")

A = 10.0
NX = 8192
NY = 8192
N_CORES = 8
MX = NX // N_CORES      # 1024 rows of x per core = 512 pairs
KH = 20                 # harmonics kept
NS = 11                 # rows with bf16 hi/lo correction (const + 5 cos + 5 sin)
NFEAT = 1 + 2 * KH      # 41 feature rows
# contraction dim padded to 64 (41 + 22 correction rows + 1 zero row):
# partition counts that aren't a multiple of 16 serialize the DMA onto a
# single engine, so keep transfers at 64 partitions
NROWS = 64

# Fourier cosine coefficients of log(I0(20 cos(pi d))) - 20 on d in [0, 1).
_B0 = -9.320623105523872
_BK = [
    7.970447139028089, -1.4358756600553582, 0.5530401566383198,
    -0.27432647869384885, 0.1547723650507224, -0.09433791302730635,
    0.060502068515108406, -0.04020530135648252, 0.027418113277826187,
    -0.01906554834357182, 0.013458315954332174, -0.009613552975863679,
    0.0069329638057468446, -0.005038947804517573, 0.003686131354141929,
    -0.00271122806102214, 0.00200343687917714, -0.0014863506699641636,
    0.00110656955440988, -0.0008263523699001975,
]

_NC_CACHE = None
LAST_EXEC_TIME_NS = None
LAST_TRACE_PATH = None


def _trig_features(s):
    """[NFEAT, n] float64 features: row 0 const, 1..KH cos, KH+1.. sin."""
    ks = np.arange(1, KH + 1, dtype=np.float64)[:, None]
    ang = 2.0 * np.pi * ks * s[None, :]
    f = np.empty((NFEAT, s.size), np.float64)
    f[0] = 1.0
    f[1 : KH + 1] = np.cos(ang)
    f[KH + 1 :] = np.sin(ang)
    return f


def _split_rows():
    nh = (NS - 1) // 2
    return np.r_[0, np.arange(1, 1 + nh), np.arange(KH + 1, KH + 1 + nh)]


def _pack_u(u64, bf16):
    """x-side [NROWS, n] bf16: hi rows, then [uh_s ; ul_s] correction rows."""
    s = _split_rows()
    uh = u64.astype(bf16)
    ul = (u64 - uh.astype(np.float64)).astype(bf16)
    out = np.zeros((NROWS, u64.shape[1]), bf16)
    out[:NFEAT] = uh
    out[NFEAT : NFEAT + NS] = uh[s]
    out[NFEAT + NS : NFEAT + 2 * NS] = ul[s]
    return out


def _pack_v(v64, bf16):
    """y-side [NROWS, n] bf16: hi rows, then [vl_s ; vh_s] partner rows."""
    s = _split_rows()
    vh = v64.astype(bf16)
    vl = (v64 - vh.astype(np.float64)).astype(bf16)
    out = np.zeros((NROWS, v64.shape[1]), bf16)
    out[:NFEAT] = vh
    out[NFEAT : NFEAT + NS] = vl[s]
    out[NFEAT + NS : NFEAT + 2 * NS] = vh[s]
    return out


def _build():
    """Build + compile the per-core Bass/Tile kernel (cached)."""
    global _NC_CACHE
    if _NC_CACHE is not None:
        return _NC_CACHE

    from concourse import bacc, mybir
    import concourse.tile as tile

    f32 = mybir.dt.float32
    f16 = mybir.dt.float16
    bf16 = mybir.dt.bfloat16

    nc = bacc.Bacc(
        "TRN2", target_bir_lowering=False, debug=False, num_devices=N_CORES
    )
    # ux: cols [0,512) even-row features, [512,1024) pair-delta features
    ux_d = nc.dram_tensor("ux", [NROWS, MX], bf16, kind="ExternalInput").ap()
    vy_d = nc.dram_tensor("vy", [NROWS, NY], bf16, kind="ExternalInput").ap()
    # out rows, block m of 128 pairs: [m*256, m*256+128) = even rows,
    # [m*256+128, (m+1)*256) = odd rows
    out_d = nc.dram_tensor("out", [MX, NY], f16, kind="ExternalOutput").ap()

    n_mb = MX // 256   # 4 pair blocks of 128 pairs
    n_g = NY // 2048   # 4 col groups

    with tile.TileContext(nc) as tc:
        with (
            tc.tile_pool(name="wpool", bufs=1) as wpool,
            tc.tile_pool(name="vpool", bufs=n_g) as vpool,
            tc.tile_pool(name="pspool", bufs=4, space="PSUM") as pspool,
            tc.tile_pool(name="opool", bufs=4) as opool,
        ):
            ux_t = wpool.tile([NROWS, MX], bf16, name="ux_t", tag="ux_t")
            nc.sync.dma_start(ux_t[:], ux_d[:])
            vys = []
            for g in range(n_g):
                vy_t = vpool.tile([NROWS, 2048], bf16, name=f"vy_{g}", tag="vy")
                vys.append(vy_t)
                nc.sync.dma_start(vy_t[:], vy_d[:, g * 2048 : (g + 1) * 2048])

            # PE warm-up on a zero tile: keeps the HAM clock at full rate so
            # the real matmul stream (starting ~1.5us in, after ux/vy0 land)
            # runs warm.
            warm_t = wpool.tile([NROWS, 640], bf16, name="warm_t", tag="warm_t")
            nc.vector.memset(warm_t[:], 0.0)
            warm_ps = pspool.tile([128, 512], f32, name="warm_ps", tag="ps")
            for _w in range(8):
                nc.tensor.matmul(
                    warm_ps[:],
                    warm_t[:, 0:128],
                    warm_t[:, 128:640],
                    start=True,
                    stop=True,
                )

            # 1024-col units with 4 PSUM tiles (2 banks each): the ev and dl
            # streams are each double-buffered, so the PE's dl matmuls for
            # unit g+1 run DURING the DVE STT of unit g instead of inside the
            # DVE->DVE critical path (with 2x2048 tiles the chain
            # DVE -> dl-matmuls -> DVE paced the whole kernel at 3.5us/2048).
            for m in range(n_mb):
                u_ev = ux_t[:, m * 128 : (m + 1) * 128]
                u_dl = ux_t[:, 512 + m * 128 : 512 + (m + 1) * 128]
                for h in range(2):
                    oute_t = opool.tile([128, 4096], f16, name=f"oe_{m}_{h}", tag="oute")
                    outo_t = opool.tile([128, 4096], f16, name=f"oo_{m}_{h}", tag="outo")
                    for gg in range(4):
                        g = 2 * h + gg // 2
                        csl = slice(gg * 1024, (gg + 1) * 1024)
                        base = (gg % 2) * 1024
                        ps_ev = pspool.tile([128, 1024], f32, name=f"pe_{m}_{h}_{gg}", tag="ps")
                        for s in range(2):
                            nc.tensor.matmul(
                                ps_ev[:, s * 512 : (s + 1) * 512], u_ev,
                                vys[g][:, base + s * 512 : base + (s + 1) * 512],
                                start=True, stop=True,
                            )
                        ps_dl = pspool.tile([128, 1024], f32, name=f"pd_{m}_{h}_{gg}", tag="ps")
                        for s in range(2):
                            nc.tensor.matmul(
                                ps_dl[:, s * 512 : (s + 1) * 512], u_dl,
                                vys[g][:, base + s * 512 : base + (s + 1) * 512],
                                start=True, stop=True,
                            )
                        # out_even = exp(L_e + 16 ln2) = 2^16 K_even (fp16)
                        nc.scalar.activation(
                            oute_t[:, csl], ps_ev[:],
                            mybir.ActivationFunctionType.Exp,
                        )
                        # out_odd = (dL + 1) * out_even
                        nc.vector.scalar_tensor_tensor(
                            outo_t[:, csl], ps_dl[:], 1.0, oute_t[:, csl],
                            mybir.AluOpType.add, mybir.AluOpType.mult,
                        )
                    rsl_e = slice(m * 256, m * 256 + 128)
                    rsl_o = slice(m * 256 + 128, (m + 1) * 256)
                    csl_h = slice(h * 4096, (h + 1) * 4096)
                    # even halves go out as soon as 2 units are done so the
                    # DMA engines never starve behind the first (m,h) block
                    for q in range(2):
                        cq = slice(h * 4096 + q * 2048, h * 4096 + (q + 1) * 2048)
                        nc.sync.dma_start(
                            out_d[rsl_e, cq], oute_t[:, q * 2048 : (q + 1) * 2048]
                        )
                    nc.sync.dma_start(out_d[rsl_o, csl_h], outo_t[:])

    nc.compile()
    _NC_CACHE = nc
    return nc


def kernel(x: np.ndarray, y: np.ndarray) -> np.ndarray:
    global LAST_EXEC_TIME_NS, LAST_TRACE_PATH
    import ml_dtypes
    from concourse import bass_utils

    bf16 = ml_dtypes.bfloat16

    xf = np.asarray(x, np.float32).reshape(-1).astype(np.float64)
    yf = np.asarray(y, np.float32).reshape(-1).astype(np.float64)

    order = np.argsort(xf, kind="stable")
    xs = xf[order]

    coefs = np.concatenate(
        [[_B0 + 16.0 * 0.6931471805599453], _BK, _BK]
    )  # 2^16 fp16 scale folded into the constant row
    ux = _trig_features(xs) * coefs[:, None]
    u_ev = _pack_u(ux[:, 0::2], bf16)                       # [101, 4096]
    u_dl64 = ux[:, 1::2] - ux[:, 0::2]
    u_dl = np.zeros((NROWS, NX // 2), bf16)
    u_dl[:NFEAT] = u_dl64.astype(bf16)

    vy = _pack_v(_trig_features(yf), bf16)                  # [101, 8192]

    nc = _build()
    nmid = MX // 2
    in_maps = [
        {
            "ux": np.concatenate(
                [u_ev[:, i * nmid : (i + 1) * nmid],
                 u_dl[:, i * nmid : (i + 1) * nmid]],
                axis=1,
            ),
            "vy": vy,
        }
        for i in range(N_CORES)
    ]
    trace = bool(os.environ.get("BESSEL_TRACE"))
    res = bass_utils.run_bass_kernel_spmd(
        nc, in_maps, core_ids=list(range(N_CORES)), trace=trace
    )
    LAST_EXEC_TIME_NS = res.exec_time_ns
    if res.instructions_and_trace is not None:
        LAST_TRACE_PATH = res.instructions_and_trace[1]

    # host: rescale by the exact 2^-16 and un-sort rows.
    # device row r (of core i): m = r//256, t = r%256
    #   t < 128  -> sorted idx i*1024 + m*256 + 2t       (even row of pair)
    #   t >= 128 -> sorted idx i*1024 + m*256 + 2(t-128)+1   (odd row)
    r = np.arange(MX)
    mblk, t = r // 256, r % 256
    sidx_local = np.where(
        t < 128, mblk * 256 + 2 * t, mblk * 256 + 2 * (t - 128) + 1
    )

    out = np.empty((NX, NY), np.float32)
    for i in range(N_CORES):
        blk = res.results[i]["out"].astype(np.float32)
        np.multiply(blk, np.float32(2.0**-16), out=blk)
        out[order[i * MX + sidx_local]] = blk
    return out


# revision 12
# speedup vs baseline: 1.0199x; 1.0199x over previous
"""Trainium2 Bass kernel for the 1-D Bessel (von Mises-like) kernel matrix:

    K[i, j] = I0(2a * cos(pi * (x_i - y_j))) * exp(-2a),   a = 10

Algorithm (pair-interpolated log-space rank-101 factorization)
--------------------------------------------------------------
log K has a rapidly converging Fourier cosine series in d = x - y:

    log K = b0 + sum_{k=1..31} b_k cos(2 pi k d)            (trunc err 1.6e-4)

so log K = U.T @ V with trig feature matrices of rank 63 (+38 bf16 hi/lo
correction rows -> K-dim 101, ONE bf16 matmul pass into fp32 PSUM).

To halve the Scalar-engine exp cost (the 1 elem/cycle/lane ACT floor), x is
sorted on host and adjacent rows are paired.  For each pair the device gets
the even row's features u(x_e) and the delta features u(x_o) - u(x_e), so
PSUM holds the even-row logs L_e and the exact pair deltas dL
(|dL| <= 0.058 on this data).  Then per 2048-col group:

    ACT:   out_even = exp(L_e)              (fp16, only HALF the rows)
    DVE:   out_odd  = (dL + 1) * out_even   (one fused scalar_tensor_tensor)

with linearization error dL^2/2 <= 1.7e-3 (gate 2e-2).  The 2^16 fp16
output scale is folded into the constant feature row; the host multiplies
by 2^-16 while un-sorting rows.  End-to-end max pointwise rel err ~2.5e-3.

Per-core busy estimates: PE 34us (128 bf16 matmuls), ACT 30us, DVE 38us,
and the 16 MiB fp16 output DMA ~41us at the 16-engine ~410 GB/s per-core
ceiling -- the kernel is output-DMA-bound by design.
"""

import os
import sys

import numpy as np

sys.path.insert(0, "/opt/trn_rl_repo")

A = 10.0
NX = 8192
NY = 8192
N_CORES = 8
MX = NX // N_CORES      # 1024 rows of x per core = 512 pairs
KH = 31                 # harmonics kept
NS = 19                 # rows with bf16 hi/lo correction (const + 9 cos + 9 sin)
NFEAT = 1 + 2 * KH      # 63 feature rows
# contraction dim padded to 128 (63 + 38 correction rows + 27 zero rows).
# Two hardware constraints force 128: partition counts that aren't a
# multiple of 16 serialize the DMA onto a single engine, and K<128
# stationaries leave half the PE array idle so the HAM activity monitor
# never boosts the clock (matmuls run 2x slow at K=64)
NROWS = 128

# Fourier cosine coefficients of log(I0(20 cos(pi d))) - 20 on d in [0, 1).
_B0 = -9.320623105523872
_BK = [
    7.970447139028089, -1.4358756600553582, 0.5530401566383198,
    -0.27432647869384885, 0.1547723650507224, -0.09433791302730635,
    0.060502068515108406, -0.04020530135648252, 0.027418113277826187,
    -0.01906554834357182, 0.013458315954332174, -0.009613552975863679,
    0.0069329638057468446, -0.005038947804517573, 0.003686131354141929,
    -0.00271122806102214, 0.00200343687917714, -0.0014863506699641636,
    0.00110656955440988, -0.0008263523699001975, 0.000618771677773785,
    -0.00046446052148687905, 0.00034939361165105417, -0.0002633536495551932,
    0.00019885898700602698, -0.0001504063999160173, 0.00011393178617259052,
    -8.642320754869491e-05, 6.564143485541695e-05, -4.991697831321222e-05,
    3.8001927162546077e-05,
]

_NC_CACHE = None
LAST_EXEC_TIME_NS = None
LAST_TRACE_PATH = None


def _trig_features(s):
    """[NFEAT, n] float64 features: row 0 const, 1..KH cos, KH+1.. sin."""
    ks = np.arange(1, KH + 1, dtype=np.float64)[:, None]
    ang = 2.0 * np.pi * ks * s[None, :]
    f = np.empty((NFEAT, s.size), np.float64)
    f[0] = 1.0
    f[1 : KH + 1] = np.cos(ang)
    f[KH + 1 :] = np.sin(ang)
    return f


def _split_rows():
    nh = (NS - 1) // 2
    return np.r_[0, np.arange(1, 1 + nh), np.arange(KH + 1, KH + 1 + nh)]


def _pack_u(u64, bf16):
    """x-side [NROWS, n] bf16: hi rows, then [uh_s ; ul_s] correction rows."""
    s = _split_rows()
    uh = u64.astype(bf16)
    ul = (u64 - uh.astype(np.float64)).astype(bf16)
    out = np.zeros((NROWS, u64.shape[1]), bf16)
    out[:NFEAT] = uh
    out[NFEAT : NFEAT + NS] = uh[s]
    out[NFEAT + NS : NFEAT + 2 * NS] = ul[s]
    return out


def _pack_v(v64, bf16):
    """y-side [NROWS, n] bf16: hi rows, then [vl_s ; vh_s] partner rows."""
    s = _split_rows()
    vh = v64.astype(bf16)
    vl = (v64 - vh.astype(np.float64)).astype(bf16)
    out = np.zeros((NROWS, v64.shape[1]), bf16)
    out[:NFEAT] = vh
    out[NFEAT : NFEAT + NS] = vl[s]
    out[NFEAT + NS : NFEAT + 2 * NS] = vh[s]
    return out


def _build():
    """Build + compile the per-core Bass/Tile kernel (cached)."""
    global _NC_CACHE
    if _NC_CACHE is not None:
        return _NC_CACHE

    from concourse import bacc, mybir
    import concourse.tile as tile

    f32 = mybir.dt.float32
    f16 = mybir.dt.float16
    bf16 = mybir.dt.bfloat16

    nc = bacc.Bacc(
        "TRN2", target_bir_lowering=False, debug=False, num_devices=N_CORES
    )
    # ux: cols [0,512) even-row features, [512,1024) pair-delta features
    ux_d = nc.dram_tensor("ux", [NROWS, MX], bf16, kind="ExternalInput").ap()
    vy_d = nc.dram_tensor("vy", [NROWS, NY], bf16, kind="ExternalInput").ap()
    # out rows, block m of 128 pairs: [m*256, m*256+128) = even rows,
    # [m*256+128, (m+1)*256) = odd rows
    out_d = nc.dram_tensor("out", [MX, NY], f16, kind="ExternalOutput").ap()

    n_mb = MX // 256   # 4 pair blocks of 128 pairs
    n_g = NY // 2048   # 4 col groups

    with tile.TileContext(nc) as tc:
        with (
            tc.tile_pool(name="wpool", bufs=1) as wpool,
            tc.tile_pool(name="vpool", bufs=n_g) as vpool,
            tc.tile_pool(name="pspool", bufs=4, space="PSUM") as pspool,
            tc.tile_pool(name="opool", bufs=4) as opool,
        ):
            ux_t = wpool.tile([NROWS, MX], bf16, name="ux_t", tag="ux_t")
            nc.sync.dma_start(ux_t[:], ux_d[:])
            vys = []
            for g in range(n_g):
                vy_t = vpool.tile([NROWS, 2048], bf16, name=f"vy_{g}", tag="vy")
                vys.append(vy_t)
                nc.sync.dma_start(vy_t[:], vy_d[:, g * 2048 : (g + 1) * 2048])

            # PE warm-up on a zero tile: keeps the HAM clock at full rate so
            # the real matmul stream (starting ~1.5us in, after ux/vy0 land)
            # runs warm.
            warm_t = wpool.tile([NROWS, 640], bf16, name="warm_t", tag="warm_t")
            nc.vector.memset(warm_t[:], 0.0)
            warm_ps = pspool.tile([128, 512], f32, name="warm_ps", tag="ps")
            for _w in range(8):
                nc.tensor.matmul(
                    warm_ps[:],
                    warm_t[:, 0:128],
                    warm_t[:, 128:640],
                    start=True,
                    stop=True,
                )

            # 1024-col units with 4 PSUM tiles (2 banks each): the ev and dl
            # streams are each double-buffered, so the PE's dl matmuls for
            # unit g+1 run DURING the DVE STT of unit g instead of inside the
            # DVE->DVE critical path (with 2x2048 tiles the chain
            # DVE -> dl-matmuls -> DVE paced the whole kernel at 3.5us/2048).
            for m in range(n_mb):
                u_ev = ux_t[:, m * 128 : (m + 1) * 128]
                u_dl = ux_t[:, 512 + m * 128 : 512 + (m + 1) * 128]
                for h in range(2):
                    oute_t = opool.tile([128, 4096], f16, name=f"oe_{m}_{h}", tag="oute")
                    outo_t = opool.tile([128, 4096], f16, name=f"oo_{m}_{h}", tag="outo")
                    for gg in range(4):
                        g = 2 * h + gg // 2
                        csl = slice(gg * 1024, (gg + 1) * 1024)
                        base = (gg % 2) * 1024
                        ps_ev = pspool.tile([128, 1024], f32, name=f"pe_{m}_{h}_{gg}", tag="ps")
                        for s in range(2):
                            nc.tensor.matmul(
                                ps_ev[:, s * 512 : (s + 1) * 512], u_ev,
                                vys[g][:, base + s * 512 : base + (s + 1) * 512],
                                start=True, stop=True,
                            )
                        ps_dl = pspool.tile([128, 1024], f32, name=f"pd_{m}_{h}_{gg}", tag="ps")
                        for s in range(2):
                            nc.tensor.matmul(
                                ps_dl[:, s * 512 : (s + 1) * 512], u_dl,
                                vys[g][:, base + s * 512 : base + (s + 1) * 512],
                                start=True, stop=True,
                            )
                        # out_even = exp(L_e + 16 ln2) = 2^16 K_even (fp16)
                        nc.scalar.activation(
                            oute_t[:, csl], ps_ev[:],
                            mybir.ActivationFunctionType.Exp,
                        )
                        # out_odd = (dL + 1) * out_even
                        nc.vector.scalar_tensor_tensor(
                            outo_t[:, csl], ps_dl[:], 1.0, oute_t[:, csl],
                            mybir.AluOpType.add, mybir.AluOpType.mult,
                        )
                    rsl_e = slice(m * 256, m * 256 + 128)
                    rsl_o = slice(m * 256 + 128, (m + 1) * 256)
                    csl_h = slice(h * 4096, (h + 1) * 4096)
                    # even halves go out as soon as 2 units are done so the
                    # DMA engines never starve behind the first (m,h) block
                    for q in range(2):
                        cq = slice(h * 4096 + q * 2048, h * 4096 + (q + 1) * 2048)
                        nc.sync.dma_start(
                            out_d[rsl_e, cq], oute_t[:, q * 2048 : (q + 1) * 2048]
                        )
                    nc.sync.dma_start(out_d[rsl_o, csl_h], outo_t[:])

    nc.compile()
    _NC_CACHE = nc
    return nc


def kernel(x: np.ndarray, y: np.ndarray) -> np.ndarray:
    global LAST_EXEC_TIME_NS, LAST_TRACE_PATH
    import ml_dtypes
    from concourse import bass_utils

    bf16 = ml_dtypes.bfloat16

    xf = np.asarray(x, np.float32).reshape(-1).astype(np.float64)
    yf = np.asarray(y, np.float32).reshape(-1).astype(np.float64)

    order = np.argsort(xf, kind="stable")
    xs = xf[order]

    coefs = np.concatenate(
        [[_B0 + 16.0 * 0.6931471805599453], _BK, _BK]
    )  # 2^16 fp16 scale folded into the constant row
    ux = _trig_features(xs) * coefs[:, None]
    u_ev = _pack_u(ux[:, 0::2], bf16)                       # [101, 4096]
    u_dl64 = ux[:, 1::2] - ux[:, 0::2]
    u_dl = np.zeros((NROWS, NX // 2), bf16)
    u_dl[:NFEAT] = u_dl64.astype(bf16)

    vy = _pack_v(_trig_features(yf), bf16)                  # [101, 8192]

    nc = _build()
    nmid = MX // 2
    in_maps = [
        {
            "ux": np.concatenate(
                [u_ev[:, i * nmid : (i + 1) * nmid],
                 u_dl[:, i * nmid : (i + 1) * nmid]],
                axis=1,
            ),
            "vy": vy,
        }
        for i in range(N_CORES)
    ]
    trace = bool(os.environ.get("BESSEL_TRACE"))
    res = bass_utils.run_bass_kernel_spmd(
        nc, in_maps, core_ids=list(range(N_CORES)), trace=trace
    )
    LAST_EXEC_TIME_NS = res.exec_time_ns
    if res.instructions_and_trace is not None:
        LAST_TRACE_PATH = res.instructions_and_trace[1]

    # host: rescale by the exact 2^-16 and un-sort rows.
    # device row r (of core i): m = r//256, t = r%256
    #   t < 128  -> sorted idx i*1024 + m*256 + 2t       (even row of pair)
    #   t >= 128 -> sorted idx i*1024 + m*256 + 2(t-128)+1   (odd row)
    r = np.arange(MX)
    mblk, t = r // 256, r % 256
    sidx_local = np.where(
        t < 128, mblk * 256 + 2 * t, mblk * 256 + 2 * (t - 128) + 1
    )

    out = np.empty((NX, NY), np.float32)
    for i in range(N_CORES):
        blk = res.results[i]["out"].astype(np.float32)
        np.multiply(blk, np.float32(2.0**-16), out=blk)
        out[order[i * MX + sidx_local]] = blk
    return out


# revision 13
# speedup vs baseline: 1.1037x; 1.0821x over previous
"""Trainium2 Bass kernel for the 1-D Bessel (von Mises-like) kernel matrix:

    K[i, j] = I0(2a * cos(pi * (x_i - y_j))) * exp(-2a),   a = 10

Algorithm (pair-interpolated log-space rank-101 factorization)
--------------------------------------------------------------
log K has a rapidly converging Fourier cosine series in d = x - y:

    log K = b0 + sum_{k=1..31} b_k cos(2 pi k d)            (trunc err 1.6e-4)

so log K = U.T @ V with trig feature matrices of rank 63 (+38 bf16 hi/lo
correction rows -> K-dim 101, ONE bf16 matmul pass into fp32 PSUM).

To halve the Scalar-engine exp cost (the 1 elem/cycle/lane ACT floor), x is
sorted on host and adjacent rows are paired.  For each pair the device gets
the even row's features u(x_e) and the delta features u(x_o) - u(x_e), so
PSUM holds the even-row logs L_e and the exact pair deltas dL
(|dL| <= 0.058 on this data).  Then per 2048-col group:

    ACT:   out_even = exp(L_e)              (fp16, only HALF the rows)
    DVE:   out_odd  = (dL + 1) * out_even   (one fused scalar_tensor_tensor)

with linearization error dL^2/2 <= 1.7e-3 (gate 2e-2).  The 2^16 fp16
output scale is folded into the constant feature row; the host multiplies
by 2^-16 while un-sorting rows.  End-to-end max pointwise rel err ~2.5e-3.

Per-core busy estimates: PE 34us (128 bf16 matmuls), ACT 30us, DVE 38us,
and the 16 MiB fp16 output DMA ~41us at the 16-engine ~410 GB/s per-core
ceiling -- the kernel is output-DMA-bound by design.
"""

import os
import sys

import numpy as np

sys.path.insert(0, "/opt/trn_rl_repo")

A = 10.0
NX = 8192
NY = 8192
N_CORES = 8
MX = NX // N_CORES      # 1024 rows of x per core = 512 pairs
KH = 31                 # harmonics kept
NS = 19                 # rows with bf16 hi/lo correction (const + 9 cos + 9 sin)
NFEAT = 1 + 2 * KH      # 63 feature rows
# contraction dim padded to 128 (63 + 38 correction rows + 27 zero rows).
# Two hardware constraints force 128: partition counts that aren't a
# multiple of 16 serialize the DMA onto a single engine, and K<128
# stationaries leave half the PE array idle so the HAM activity monitor
# never boosts the clock (matmuls run 2x slow at K=64)
NROWS = 128

# Fourier cosine coefficients of log(I0(20 cos(pi d))) - 20 on d in [0, 1).
_B0 = -9.320623105523872
_BK = [
    7.970447139028089, -1.4358756600553582, 0.5530401566383198,
    -0.27432647869384885, 0.1547723650507224, -0.09433791302730635,
    0.060502068515108406, -0.04020530135648252, 0.027418113277826187,
    -0.01906554834357182, 0.013458315954332174, -0.009613552975863679,
    0.0069329638057468446, -0.005038947804517573, 0.003686131354141929,
    -0.00271122806102214, 0.00200343687917714, -0.0014863506699641636,
    0.00110656955440988, -0.0008263523699001975, 0.000618771677773785,
    -0.00046446052148687905, 0.00034939361165105417, -0.0002633536495551932,
    0.00019885898700602698, -0.0001504063999160173, 0.00011393178617259052,
    -8.642320754869491e-05, 6.564143485541695e-05, -4.991697831321222e-05,
    3.8001927162546077e-05,
]

_NC_CACHE = None
LAST_EXEC_TIME_NS = None
LAST_TRACE_PATH = None


def _trig_features(s):
    """[NFEAT, n] float64 features: row 0 const, 1..KH cos, KH+1.. sin."""
    ks = np.arange(1, KH + 1, dtype=np.float64)[:, None]
    ang = 2.0 * np.pi * ks * s[None, :]
    f = np.empty((NFEAT, s.size), np.float64)
    f[0] = 1.0
    f[1 : KH + 1] = np.cos(ang)
    f[KH + 1 :] = np.sin(ang)
    return f


def _split_rows():
    nh = (NS - 1) // 2
    return np.r_[0, np.arange(1, 1 + nh), np.arange(KH + 1, KH + 1 + nh)]


def _pack_u(u64, bf16):
    """x-side [NROWS, n] bf16: hi rows, then [uh_s ; ul_s] correction rows."""
    s = _split_rows()
    uh = u64.astype(bf16)
    ul = (u64 - uh.astype(np.float64)).astype(bf16)
    out = np.zeros((NROWS, u64.shape[1]), bf16)
    out[:NFEAT] = uh
    out[NFEAT : NFEAT + NS] = uh[s]
    out[NFEAT + NS : NFEAT + 2 * NS] = ul[s]
    return out


def _pack_v(v64, bf16):
    """y-side [NROWS, n] bf16: hi rows, then [vl_s ; vh_s] partner rows."""
    s = _split_rows()
    vh = v64.astype(bf16)
    vl = (v64 - vh.astype(np.float64)).astype(bf16)
    out = np.zeros((NROWS, v64.shape[1]), bf16)
    out[:NFEAT] = vh
    out[NFEAT : NFEAT + NS] = vl[s]
    out[NFEAT + NS : NFEAT + 2 * NS] = vh[s]
    return out


def _build():
    """Build + compile the per-core Bass/Tile kernel (cached)."""
    global _NC_CACHE
    if _NC_CACHE is not None:
        return _NC_CACHE

    from concourse import bacc, mybir
    import concourse.tile as tile

    f32 = mybir.dt.float32
    f16 = mybir.dt.float16
    bf16 = mybir.dt.bfloat16

    nc = bacc.Bacc(
        "TRN2", target_bir_lowering=False, debug=False, num_devices=N_CORES
    )
    # ux: cols [0,512) even-row features, [512,1024) pair-delta features
    ux_d = nc.dram_tensor("ux", [NROWS, MX], bf16, kind="ExternalInput").ap()
    vy_d = nc.dram_tensor("vy", [NROWS, NY], bf16, kind="ExternalInput").ap()
    # out rows, block m of 128 pairs: [m*256, m*256+128) = even rows,
    # [m*256+128, (m+1)*256) = odd rows
    out_d = nc.dram_tensor("out", [MX, NY], f16, kind="ExternalOutput").ap()

    n_mb = MX // 256   # 4 pair blocks of 128 pairs
    n_g = NY // 2048   # 4 col groups

    with tile.TileContext(nc) as tc:
        with (
            tc.tile_pool(name="wpool", bufs=1) as wpool,
            tc.tile_pool(name="vpool", bufs=n_g) as vpool,
            tc.tile_pool(name="pspool", bufs=4, space="PSUM") as pspool,
            tc.tile_pool(name="opool", bufs=4) as opool,
        ):
            ux_t = wpool.tile([NROWS, MX], bf16, name="ux_t", tag="ux_t")
            nc.sync.dma_start(ux_t[:], ux_d[:])
            vys = []
            for g in range(n_g):
                vy_t = vpool.tile([NROWS, 2048], bf16, name=f"vy_{g}", tag="vy")
                vys.append(vy_t)
                nc.sync.dma_start(vy_t[:], vy_d[:, g * 2048 : (g + 1) * 2048])

            # PE warm-up on a zero tile: keeps the HAM clock at full rate so
            # the real matmul stream (starting ~1.5us in, after ux/vy0 land)
            # runs warm.
            warm_t = wpool.tile([NROWS, 640], bf16, name="warm_t", tag="warm_t")
            nc.vector.memset(warm_t[:], 0.0)
            warm_ps = pspool.tile([128, 512], f32, name="warm_ps", tag="ps")
            for _w in range(8):
                nc.tensor.matmul(
                    warm_ps[:],
                    warm_t[:, 0:128],
                    warm_t[:, 128:640],
                    start=True,
                    stop=True,
                )

            # 1024-col units with 4 PSUM tiles (2 banks each): the ev and dl
            # streams are each double-buffered, so the PE's dl matmuls for
            # unit g+1 run DURING the DVE STT of unit g instead of inside the
            # DVE->DVE critical path (with 2x2048 tiles the chain
            # DVE -> dl-matmuls -> DVE paced the whole kernel at 3.5us/2048).
            for m in range(n_mb):
                u_ev = ux_t[:, m * 128 : (m + 1) * 128]
                u_dl = ux_t[:, 512 + m * 128 : 512 + (m + 1) * 128]
                for h in range(2):
                    oute_t = opool.tile([128, 4096], f16, name=f"oe_{m}_{h}", tag="oute")
                    outo_t = opool.tile([128, 4096], f16, name=f"oo_{m}_{h}", tag="outo")
                    for gg in range(4):
                        g = 2 * h + gg // 2
                        csl = slice(gg * 1024, (gg + 1) * 1024)
                        base = (gg % 2) * 1024
                        ps_ev = pspool.tile([128, 1024], f32, name=f"pe_{m}_{h}_{gg}", tag="ps")
                        for s in range(2):
                            nc.tensor.matmul(
                                ps_ev[:, s * 512 : (s + 1) * 512], u_ev,
                                vys[g][:, base + s * 512 : base + (s + 1) * 512],
                                start=True, stop=True,
                            )
                        ps_dl = pspool.tile([128, 1024], f32, name=f"pd_{m}_{h}_{gg}", tag="ps")
                        for s in range(2):
                            nc.tensor.matmul(
                                ps_dl[:, s * 512 : (s + 1) * 512], u_dl,
                                vys[g][:, base + s * 512 : base + (s + 1) * 512],
                                start=True, stop=True,
                            )
                        # out_even = exp(L_e + 16 ln2) = 2^16 K_even (fp16)
                        nc.scalar.activation(
                            oute_t[:, csl], ps_ev[:],
                            mybir.ActivationFunctionType.Exp,
                        )
                        # out_odd = (dL + 1) * out_even
                        nc.vector.scalar_tensor_tensor(
                            outo_t[:, csl], ps_dl[:], 1.0, oute_t[:, csl],
                            mybir.AluOpType.add, mybir.AluOpType.mult,
                        )
                    rsl_e = slice(m * 256, m * 256 + 128)
                    rsl_o = slice(m * 256 + 128, (m + 1) * 256)
                    csl_h = slice(h * 4096, (h + 1) * 4096)
                    nc.sync.dma_start(out_d[rsl_e, csl_h], oute_t[:])
                    nc.sync.dma_start(out_d[rsl_o, csl_h], outo_t[:])

            for _w in range(44):
                nc.tensor.matmul(
                    warm_ps[:],
                    warm_t[:, 0:128],
                    warm_t[:, 128:640],
                    start=True,
                    stop=True,
                )

    nc.compile()
    _NC_CACHE = nc
    return nc


def kernel(x: np.ndarray, y: np.ndarray) -> np.ndarray:
    global LAST_EXEC_TIME_NS, LAST_TRACE_PATH
    import ml_dtypes
    from concourse import bass_utils

    bf16 = ml_dtypes.bfloat16

    xf = np.asarray(x, np.float32).reshape(-1).astype(np.float64)
    yf = np.asarray(y, np.float32).reshape(-1).astype(np.float64)

    order = np.argsort(xf, kind="stable")
    xs = xf[order]

    coefs = np.concatenate(
        [[_B0 + 16.0 * 0.6931471805599453], _BK, _BK]
    )  # 2^16 fp16 scale folded into the constant row
    ux = _trig_features(xs) * coefs[:, None]
    u_ev = _pack_u(ux[:, 0::2], bf16)                       # [101, 4096]
    u_dl64 = ux[:, 1::2] - ux[:, 0::2]
    u_dl = np.zeros((NROWS, NX // 2), bf16)
    u_dl[:NFEAT] = u_dl64.astype(bf16)

    vy = _pack_v(_trig_features(yf), bf16)                  # [101, 8192]

    nc = _build()
    nmid = MX // 2
    in_maps = [
        {
            "ux": np.concatenate(
                [u_ev[:, i * nmid : (i + 1) * nmid],
                 u_dl[:, i * nmid : (i + 1) * nmid]],
                axis=1,
            ),
            "vy": vy,
        }
        for i in range(N_CORES)
    ]
    trace = bool(os.environ.get("BESSEL_TRACE"))
    res = bass_utils.run_bass_kernel_spmd(
        nc, in_maps, core_ids=list(range(N_CORES)), trace=trace
    )
    LAST_EXEC_TIME_NS = res.exec_time_ns
    if res.instructions_and_trace is not None:
        LAST_TRACE_PATH = res.instructions_and_trace[1]

    # host: rescale by the exact 2^-16 and un-sort rows.
    # device row r (of core i): m = r//256, t = r%256
    #   t < 128  -> sorted idx i*1024 + m*256 + 2t       (even row of pair)
    #   t >= 128 -> sorted idx i*1024 + m*256 + 2(t-128)+1   (odd row)
    r = np.arange(MX)
    mblk, t = r // 256, r % 256
    sidx_local = np.where(
        t < 128, mblk * 256 + 2 * t, mblk * 256 + 2 * (t - 128) + 1
    )

    out = np.empty((NX, NY), np.float32)
    for i in range(N_CORES):
        blk = res.results[i]["out"].astype(np.float32)
        np.multiply(blk, np.float32(2.0**-16), out=blk)
        out[order[i * MX + sidx_local]] = blk
    return out
